# revision 8
# baseline (speedup 1.0000x reference)
"""Binarized conv2d kernel for Trainium2, SPMD over 8 NeuronCores.

Math (forward-value equivalent of the reference):
    real_w  = sum_k RV[k] * weights[k]          # [256,256,3,3]
    scale   = mean(|real_w|, axis=(1,2,3))      # per out-channel
    out     = conv2d(sign(x), sign(real_w), pad=1) * (scale * alpha)

sign(x) and sign(real_w) are {-1,0,+1} which are exact in fp8e4, so the conv
is computed with fp8 DoubleRow matmuls (exact integer accumulation in fp32
PSUM) and the per-channel scale*alpha is applied on PSUM evacuation.

Weight prep runs on the PE as mix-transpose matmuls: lhsT = w_k chunk
[co,ci] (fp32 for the first ci-half of h0 — doubles as the HAM clock-gate
warmup — bf16 with rv folded in elsewhere), rhs = (rv_k-scaled) identity,
accumulated over the K replicas in PSUM. The PSUM chunk [ci,co] is already
transposed for the conv lhsT, so ACT signs it straight into the fp8 wT and
DVE abs-accumulates it for the scale factor. This removes the old DVE
mix + separate PE transpose pass and starts real PE work as soon as the
first weight chunk lands.

Sharding: data-parallel over batch, 4 images per core; weights/RV/alpha
replicated. No collectives.
"""

import numpy as np
from contextlib import ExitStack

import concourse.bass as bass
import concourse.bacc as bacc
import concourse.tile as tile
from concourse import mybir
from concourse.bass_utils import run_bass_kernel_spmd
from concourse.masks import make_identity

# Problem shapes (hardcoded per contract)
B, C, H, W = 32, 256, 56, 56
K, KS = 4, 3
NCORES = 8
BL = B // NCORES            # images per core

PW = W + 1                  # packed plane width 57 (shared L/R pad column)
NPR = H + 2                 # plane rows incl. top/bottom pad
PLANE = NPR * PW            # 3306
PL = 3312                   # plane stride (>= GO+PLANE+guard, multiple of 16)
GO = 1                      # guard offset: plane data starts at elem 1
RPC = 8                     # rows per chunk
CHUNK = RPC * PW            # 456 elems per matmul (one PSUM bank)
NCHUNK = H // RPC           # 7 chunks: psum tile A gets 4, tile B gets 3
PT_CHUNKS = (4, 3)
CIH = C // 128              # 2 ci halves
COH = C // 128              # 2 co halves
TAPS = KS * KS              # 9
HCI = (C // CIH) * TAPS     # 1152 weight cols per (k, ci-half)
XSPL = 36                   # x DMA/sign row split (rows 0..35 / 36..55)

F32 = mybir.dt.float32
FP8 = mybir.dt.float8e4
BF16 = mybir.dt.bfloat16

_cache = {}


def _build():
    nc = bacc.Bacc("TRN2", target_bir_lowering=False, debug=False,
                   num_devices=NCORES)
    x_d = nc.dram_tensor("x", [BL, C, H, W], F32, kind="ExternalInput")
    w_d = nc.dram_tensor("weights", [K, C, C, KS, KS], F32, kind="ExternalInput")
    rv_d = nc.dram_tensor("RV", [K + 1], F32, kind="ExternalInput")
    al_d = nc.dram_tensor("alpha", [C, 1, 1], F32, kind="ExternalInput")
    # bf16 outputs: halves write traffic and the end-of-kernel DMA drain
    # (upcast to f32 on the host); quantization ~2.5e-03 rel, far inside
    # the 2e-2 gate
    o_d = nc.dram_tensor("out", [BL, C, H, W], BF16, kind="ExternalOutput")

    with tile.TileContext(nc) as tc, ExitStack() as ctx:
        consts = ctx.enter_context(tc.tile_pool(name="consts", bufs=1))
        wstage = ctx.enter_context(tc.tile_pool(name="wstage", bufs=16))
        wconv = ctx.enter_context(tc.tile_pool(name="wconv", bufs=8))
        wabs = ctx.enter_context(tc.tile_pool(name="wabs", bufs=4))
        xin = ctx.enter_context(tc.tile_pool(name="xin", bufs=4))
        xpads = ctx.enter_context(tc.tile_pool(name="xpads", bufs=1))
        outp = ctx.enter_context(tc.tile_pool(name="outp", bufs=2))

        # --- tiny constant loads on the ACT HWDGE ring (keeps the sync
        # ring free for the big weight/x DMAs) -----------------------------
        rv = consts.tile([128, K], F32, tag="rv")
        rv_src = bass.AP(tensor=rv_d.ap().tensor, offset=0,
                         ap=[[0, 128], [1, K]])
        nc.scalar.dma_start(out=rv, in_=rv_src)
        alpha_sb = []
        for h in range(COH):
            t = consts.tile([128, 1], F32, tag=f"alpha{h}")
            nc.scalar.dma_start(out=t,
                                in_=al_d.ap()[h * 128:(h + 1) * 128, 0, :])
            alpha_sb.append(t)

        # identities / ones for the PE-side weight mix
        ident_f = consts.tile([128, 128], F32, tag="identf")
        make_identity(nc, ident_f)
        ident_b = consts.tile([128, 128], BF16, tag="identb")
        make_identity(nc, ident_b)
        ones_f = consts.tile([128, 1], F32, tag="ones")
        nc.vector.memset(ones_f, 1.0)
        # rv-scaled f32 identities (exact-f32 mix path for h0/ci0)
        rvI = []
        for k in range(K):
            t = consts.tile([128, 128], F32, tag=f"rvI{k}")
            nc.vector.scalar_tensor_tensor(
                t, ident_f, rv[:, k:k + 1], ident_f,
                mybir.AluOpType.mult, mybir.AluOpType.bypass)
            rvI.append(t)

        # Padded planes (PW=57: one shared pad column between rows).
        # Zero only the pads on DVE; interior is overwritten by sign(x).
        xpad = []
        for i in range(2):
            t = xpads.tile([128, CIH, PL], FP8, tag=f"xpad{i}",
                           name=f"xpad{i}")
            for s in range(CIH):
                pl = t[:, s, :]
                # guard + top pad row
                nc.vector.memset(pl[:, 0:GO + PW], 0.0)
                # shared pad column x=56 of every plane row
                nc.vector.memset(
                    pl[:, GO:GO + PLANE].rearrange(
                        "p (y x) -> p y x", x=PW)[:, :, PW - 1:PW], 0.0)
                # bottom pad row + tail guard
                nc.vector.memset(pl[:, GO + (NPR - 1) * PW:PL], 0.0)
            xpad.append(t)

        wT = consts.tile([128, TAPS, COH, CIH, 128], FP8, tag="wT")
        scale_alpha = [consts.tile([128, 1], F32, tag=f"sa{h}", name=f"sa{h}")
                       for h in range(COH)]
        absacc = [consts.tile([128, 128], F32, tag=f"ab{c}", name=f"ab{c}")
                  for c in range(CIH)]

        # --- weight DMA / convert helpers ---------------------------------
        def dma_w(h, c):
            wks = []
            for k in range(K):
                wk = wstage.tile([128, HCI], F32, tag="wsb", name="wk")
                wks.append(wk)
                nc.sync.dma_start(
                    out=wk,
                    in_=w_d.ap()[k, h * 128:(h + 1) * 128,
                                 c * (C // CIH):(c + 1) * (C // CIH)]
                    .rearrange("p c a b -> p (c a b)"))
            return wks

        def convert(wks):
            # f32 -> bf16 with rv folded in (f32 multiply, then round) and
            # the (ci, tap) -> (tap, ci) rearrange for contiguous lhsT taps
            wkbs = []
            for k in range(K):
                wkb = wconv.tile([128, TAPS, 128], BF16, tag="wkb",
                                 name="wkb")
                src = wks[k].rearrange("p (c t) -> p t c", t=TAPS)
                nc.vector.scalar_tensor_tensor(
                    wkb, src, rv[:, k:k + 1], src,
                    mybir.AluOpType.mult, mybir.AluOpType.bypass)
                wkbs.append(wkb)
            return wkbs

        # --- mix one (tap, ci-half) chunk on the PE into a psum slot ------
        def mix_slot(slot, h, c, tap, wks, fp32):
            for k in range(K):
                if fp32:
                    lhsT = wks[k].rearrange(
                        "p (c t) -> p c t", t=TAPS)[:, :, tap]
                    rhs = rvI[k]
                else:
                    lhsT = wks[k][:, tap, :]
                    rhs = ident_b
                nc.tensor.matmul(slot, lhsT, rhs,
                                 start=(k == 0), stop=(k == K - 1))

        def evac_slot(slot, h, c, tap, first):
            nc.scalar.sign(wT[:, tap, h, c, :], slot)
            # |mix| via ACT (abs_max is not a valid DVE TensorScalarPtr op),
            # then DVE accumulates into absacc for the per-co scale
            ab = wabs.tile([128, 128], F32, tag="wab", name="wab")
            nc.scalar.activation(ab, slot, mybir.ActivationFunctionType.Abs,
                                 bias=0.0, scale=1.0)
            nc.vector.scalar_tensor_tensor(
                absacc[c], ab, 0.0, ident_f if first else absacc[c],
                mybir.AluOpType.bypass,
                mybir.AluOpType.bypass if first else mybir.AluOpType.add)

        # rounds: list of (psum_tag, width, slots) where slots are
        # (tap, ci-half) pairs; fp32_ci0 selects the exact-f32 PE path
        def prep_rounds(h, cpsum, wk_by_c, wkb_by_c, rounds, fp32_ci0):
            for tag, width, slots in rounds:
                tp = cpsum.tile([128, width], F32, tag=tag, bufs=1,
                                name=f"m{tag}")
                for i, (tap, c) in enumerate(slots):
                    fp32 = fp32_ci0 and c == 0
                    mix_slot(tp[:, i * 128:(i + 1) * 128], h, c, tap,
                             wk_by_c[c] if fp32 else wkb_by_c[c], fp32)
                for i, (tap, c) in enumerate(slots):
                    evac_slot(tp[:, i * 128:(i + 1) * 128], h, c, tap,
                              first=(tap == 0))

        def scale_half(h, cpsum):
            # sum absacc over ci partitions via two tiny f32 ones-matmuls
            tp = cpsum.tile([128, 512], F32, tag="tps", bufs=1, name="sc")
            sc = tp[:, 0:1]
            for c in range(CIH):
                nc.tensor.matmul(sc, absacc[c], ones_f,
                                 start=(c == 0), stop=(c == CIH - 1))
            nc.vector.scalar_tensor_tensor(
                scale_alpha[h], sc, 1.0 / (C * TAPS), alpha_sb[h],
                mybir.AluOpType.mult, mybir.AluOpType.mult)

        # --- load + sign one image into its padded plane ------------------
        # x DMA split at row XSPL so sign (and the first conv) can start
        # before the whole image lands.
        def load(b, part=None):
            # upper rows of both ci-halves first: the first conv ptile only
            # needs rows < XSPL, so it must not queue behind the lower rows
            tiles = []
            for (r0, r1) in ((0, XSPL), (XSPL, H)):
                if part == "lower" and r0 == 0:
                    continue
                if part == "upper" and r0 != 0:
                    continue
                for s in range(CIH):
                    xs = xin.tile([128, (r1 - r0) * W], F32, tag="xsb",
                                  name="xsb")
                    nc.sync.dma_start(
                        out=xs,
                        in_=x_d.ap()[b, s * 128:(s + 1) * 128, r0:r1]
                        .rearrange("p a b -> p (a b)"))
                    tiles.append((s, r0, r1, xs))
            return tiles

        def sign(b, tiles):
            xp = xpad[b % 2]
            for (s, r0, r1, xs) in tiles:
                dst = xp[:, s, GO + (r0 + 1) * PW:GO + (r1 + 1) * PW] \
                    .rearrange("p (y x) -> p y x", x=PW)[:, :, 0:W]
                nc.scalar.sign(dst, xs.rearrange("p (y x) -> p y x", x=W))

        # --- conv for one (image, co-half) --------------------------------
        def conv(b, h, cpsum, tail=False):
            xp = xpad[b % 2]
            osb = outp.tile([128, H * W], BF16, tag="osb", name="osb")
            c0 = 0
            for t, nch in enumerate(PT_CHUNKS):
                ps = cpsum.tile([128, nch * 512], F32, tag=f"ps{t}", bufs=1,
                                name=f"ps{t}")
                for tap in range(TAPS):
                    dy, dx = tap // KS - 1, tap % KS - 1
                    lhsT = wT[:, tap, h, :, :]
                    for j in range(nch):
                        c = c0 + j
                        off = GO + (RPC * c + dy + 1) * PW + dx
                        nc.tensor.matmul(
                            ps[:, j * 512:j * 512 + CHUNK], lhsT,
                            xp[:, :, off:off + CHUNK],
                            start=(tap == 0), stop=(tap == TAPS - 1),
                            perf_mode=mybir.MatmulPerfMode.DoubleRow)
                # the very last ptile of the kernel drains in single-chunk
                # pieces so the final evac and output DMA pipeline
                pieces = ([(j, j + 1) for j in range(nch)] if (tail and t == 1)
                          else [(0, nch)])
                for ja, jb in pieces:
                    src = ps.rearrange("p (c e) -> p c e", e=512)[
                        :, ja:jb, 0:CHUNK].rearrange(
                        "p c (r x) -> p c r x", x=PW)[:, :, :, 0:W]
                    dst = osb.rearrange("p (y x) -> p y x", x=W)[
                        :, (c0 + ja) * RPC:(c0 + jb) * RPC, :].rearrange(
                        "p (c r) x -> p c r x", r=RPC)
                    # PSUM evacuation on ACT with fused scale*alpha
                    nc.scalar.activation(dst, src,
                                         mybir.ActivationFunctionType.Copy,
                                         bias=0.0, scale=scale_alpha[h])
                    # per-piece output DMA on the ACT ring
                    nc.scalar.dma_start(
                        out=o_d.ap()[b, h * 128:(h + 1) * 128,
                                     (c0 + ja) * RPC:(c0 + jb) * RPC, :]
                        .rearrange("p a b -> p (a b)"),
                        in_=osb[:, (c0 + ja) * RPC * W:(c0 + jb) * RPC * W])
                c0 += nch

        # --- schedule ------------------------------------------------------
        with tc.tile_pool(name="cpsum", bufs=1, space="PSUM") as cpsum:
            # DMA ring order: w-h0/ci0, x0 upper rows, w-h0/ci1, x0 lower
            # rows, then h1 weights and the image stream.
            wk_h0 = [dma_w(0, 0)]
            xt0u = load(0, part="upper")
            wk_h0.append(dma_w(0, 1))
            xt0l = load(0, part="lower")
            wkb_h0 = [None, convert(wk_h0[1])]
            sign(0, xt0u)
            # h0 mix: ci0 in exact f32 (4x-slower matmuls double as the HAM
            # clock-gate warmup while ci1 still streams in), ci1 in bf16
            rounds_h0 = [
                ("tps", 512, [(t, 0) for t in range(0, 4)]),
                ("ps1", 3 * 512, [(t, 0) for t in range(4, 9)]
                 + [(t, 1) for t in range(0, 7)]),
                ("tps", 512, [(t, 1) for t in range(7, 9)]),
            ]
            prep_rounds(0, cpsum, wk_h0, wkb_h0, rounds_h0, fp32_ci0=True)
            scale_half(0, cpsum)
            sign(0, xt0l)

            # h1 weights land while conv(0,0) streams; mix rounds are
            # emitted between its ptiles so the in-order PE stays dense
            wk_h1 = [dma_w(1, 0), dma_w(1, 1)]
            wkb_h1 = [convert(wk_h1[0]), convert(wk_h1[1])]
            conv(0, 0, cpsum)
            rounds_h1 = [
                ("tps", 512, [(t, c) for t in range(0, 2) for c in range(2)]),
                ("tps", 512, [(t, c) for t in range(2, 4) for c in range(2)]),
                ("tps", 512, [(t, c) for t in range(4, 6) for c in range(2)]),
                ("tps", 512, [(t, c) for t in range(6, 8) for c in range(2)]),
                ("tps", 512, [(8, c) for c in range(2)]),
            ]
            prep_rounds(1, cpsum, wk_h1, wkb_h1, rounds_h1, fp32_ci0=False)
            scale_half(1, cpsum)
            xt1 = load(1)
            sign(1, xt1)
            conv(0, 1, cpsum)
            for b in range(1, BL):
                if b + 1 < BL:
                    xt = load(b + 1)   # prefetch ahead of this image's evacs
                    sign(b + 1, xt)
                conv(b, 0, cpsum)
                conv(b, 1, cpsum, tail=(b == BL - 1))
    nc.compile()
    return nc


def _get_nc():
    if "nc" not in _cache:
        _cache["nc"] = _build()
    return _cache["nc"]


def run(inputs, trace=False):
    nc = _get_nc()
    x = np.ascontiguousarray(inputs["x"], dtype=np.float32)
    in_maps = [
        {
            "x": x[c * BL:(c + 1) * BL],
            "weights": np.ascontiguousarray(inputs["weights"], np.float32),
            "RV": np.ascontiguousarray(inputs["RV"], np.float32),
            "alpha": np.ascontiguousarray(inputs["alpha"], np.float32),
        }
        for c in range(NCORES)
    ]
    res = run_bass_kernel_spmd(nc, in_maps, core_ids=list(range(NCORES)),
                               trace=trace)
    out = np.concatenate([np.asarray(r["out"]).astype(np.float32)
                          for r in res.results], axis=0)
    return out, res


def kernel(**inputs) -> np.ndarray:
    out, _ = run(inputs, trace=False)
    return out


# revision 12
# speedup vs baseline: 1.1695x; 1.1695x over previous
"""Binarized conv2d kernel for Trainium2, SPMD over 8 NeuronCores.

Math (forward-value equivalent of the reference):
    real_w  = sum_k RV[k] * weights[k]          # [256,256,3,3], exact fp32
    scale   = mean(|real_w|, axis=(1,2,3))      # per out-channel
    out     = conv2d(sign(x), sign(real_w), pad=1) * (scale * alpha)

sign(x) and sign(real_w) are {-1,0,+1} which are exact in fp8e4, so the conv
is computed with fp8 DoubleRow matmuls (exact integer accumulation in fp32
PSUM) and the per-channel scale*alpha is applied on PSUM evacuation.

The weight mix stays in exact fp32 on DVE (a bf16/PE-mix variant flips
~0.1% of weight signs and blows the error budget); fp8 sign-weights are
transposed on the PE via identity matmuls. Startup is optimized over the
original pipeline:
  - rv/alpha land via single-partition DMA + on-chip broadcast (GpSimd /
    tiny PE matmuls) instead of a 128-descriptor broadcast DMA (~14us).
  - weight DMAs precede the x stream on the ring; x is split at row 36 so
    sign(x) and the first conv ptile don't wait for the whole image.
  - plane width 57 (shared L/R pad column) instead of 58: 2% less matmul
    free-dim work.
  - fp32 warmup matmuls on freshly-landed weight chunks open the HAM
    clock gate before the first conv.
  - the final ptile drains in single-chunk pieces to pipeline the tail.

Sharding: data-parallel over batch, 4 images per core; weights/RV/alpha
replicated. No collectives.
"""

import numpy as np
from contextlib import ExitStack

import concourse.bass as bass
import concourse.bacc as bacc
import concourse.tile as tile
from concourse import mybir
from concourse.bass_utils import run_bass_kernel_spmd
from concourse.masks import make_identity

# Problem shapes (hardcoded per contract)
B, C, H, W = 32, 256, 56, 56
K, KS = 4, 3
NCORES = 8
BL = B // NCORES            # images per core

PW = W + 1                  # packed plane width 57 (shared L/R pad column)
NPR = H + 2                 # plane rows incl. top/bottom pad
PLANE = NPR * PW            # 3306
PL = 3312                   # plane stride (>= GO+PLANE+guard, multiple of 16)
GO = 1                      # guard offset: plane data starts at elem 1
RPC = 8                     # rows per chunk
CHUNK = RPC * PW            # 456 elems per matmul (one PSUM bank)
NCHUNK = H // RPC           # 7 chunks: psum tile A gets 4, tile B gets 3
PT_CHUNKS = (4, 3)
CIH = C // 128              # 2 ci halves
COH = C // 128              # 2 co halves
TAPS = KS * KS              # 9
HCI = (C // CIH) * TAPS     # 1152 weight cols per (k, ci-half)
XSPL = 36                   # x DMA/sign row split (rows 0..35 / 36..55)

F32 = mybir.dt.float32
FP8 = mybir.dt.float8e4
BF16 = mybir.dt.bfloat16

_cache = {}


def _build():
    nc = bacc.Bacc("TRN2", target_bir_lowering=False, debug=False,
                   num_devices=NCORES)
    x_d = nc.dram_tensor("x", [BL, C, H, W], F32, kind="ExternalInput")
    w_d = nc.dram_tensor("weights", [K, C, C, KS, KS], F32, kind="ExternalInput")
    rv_d = nc.dram_tensor("RV", [K + 1], F32, kind="ExternalInput")
    al_d = nc.dram_tensor("alpha", [C, 1, 1], F32, kind="ExternalInput")
    # bf16 outputs: halves write traffic and the end-of-kernel DMA drain
    # (upcast to f32 on the host); quantization ~2.5e-03 rel, far inside
    # the 2e-2 gate
    o_d = nc.dram_tensor("out", [BL, C, H, W], BF16, kind="ExternalOutput")

    with tile.TileContext(nc) as tc, ExitStack() as ctx:
        consts = ctx.enter_context(tc.tile_pool(name="consts", bufs=1))
        wstage = ctx.enter_context(tc.tile_pool(name="wstage", bufs=16))
        wwork = ctx.enter_context(tc.tile_pool(name="wwork", bufs=2))
        xin = ctx.enter_context(tc.tile_pool(name="xin", bufs=4))
        xpads = ctx.enter_context(tc.tile_pool(name="xpads", bufs=1))
        outp = ctx.enter_context(tc.tile_pool(name="outp", bufs=2))

        # --- rv / alpha: single-partition DMA + on-chip broadcast ---------
        rv1 = consts.tile([128, K], F32, tag="rv1")
        nc.scalar.dma_start(out=rv1[0:1, :], in_=rv_d.ap()[0:K])
        al1 = consts.tile([128, C], F32, tag="al1")
        nc.scalar.dma_start(out=al1[0:1, :],
                            in_=al_d.ap().rearrange("c a b -> (c a b)"))
        rv = consts.tile([128, K], F32, tag="rv")
        nc.gpsimd.partition_broadcast(rv, rv1[0:1, :])

        ident_8 = consts.tile([128, 128], FP8, tag="ident8")
        make_identity(nc, ident_8)
        ones_f = consts.tile([128, 1], F32, tag="ones")
        nc.vector.memset(ones_f, 1.0)

        # Padded planes (PW=57: one shared pad column between rows).
        # Zero only the pads on DVE; interior is overwritten by sign(x).
        xpad = []
        for i in range(2):
            t = xpads.tile([128, CIH, PL], FP8, tag=f"xpad{i}",
                           name=f"xpad{i}")
            for s in range(CIH):
                pl = t[:, s, :]
                nc.vector.memset(pl[:, 0:GO + PW], 0.0)
                nc.vector.memset(
                    pl[:, GO:GO + PLANE].rearrange(
                        "p (y x) -> p y x", x=PW)[:, :, PW - 1:PW], 0.0)
                nc.vector.memset(pl[:, GO + (NPR - 1) * PW:PL], 0.0)
            xpad.append(t)

        wT = consts.tile([128, TAPS, COH, CIH, 128], FP8, tag="wT")
        scale_alpha = [consts.tile([128, 1], F32, tag=f"sa{h}", name=f"sa{h}")
                       for h in range(COH)]
        alpha_sb = [consts.tile([128, 1], F32, tag=f"alpha{h}",
                                name=f"alpha{h}")
                    for h in range(COH)]

        def bcast_alpha(cpsum):
            # alpha [1,256] -> per-partition [128,1] per half via a tiny
            # 1-contraction PE transpose matmul
            tp = cpsum.tile([128, 512], F32, tag="tps", bufs=1, name="bca")
            for h in range(COH):
                nc.tensor.matmul(tp[:, h:h + 1],
                                 al1[0:1, h * 128:(h + 1) * 128],
                                 ones_f[0:1, :], start=True, stop=True)
            for h in range(COH):
                nc.scalar.copy(alpha_sb[h], tp[:, h:h + 1])

        # --- weight DMA -----------------------------------------------------
        def dma_w(h, c):
            wks = []
            for k in range(K):
                wk = wstage.tile([128, HCI], F32, tag="wsb", name="wk")
                wks.append(wk)
                nc.sync.dma_start(
                    out=wk,
                    in_=w_d.ap()[k, h * 128:(h + 1) * 128,
                                 c * (C // CIH):(c + 1) * (C // CIH)]
                    .rearrange("p c a b -> p (c a b)"))
            return wks

        # fp32 warmup matmuls on a landed weight chunk: keeps the PE busy
        # so the HAM clock gate opens before (and stays open until) the
        # first conv matmuls
        def warm(cpsum, wk, n):
            for i in range(n):
                wtp = cpsum.tile([128, 512], F32, tag="tps", bufs=1,
                                 name="warm")
                nc.tensor.matmul(wtp[:, 0:464], wk[:, 0:128], wk[:, 0:464],
                                 start=True, stop=True)

        # --- exact-f32 weight mix on DVE + sign per ci-half ---------------
        def prep_half(h, wks_by_c):
            wmix = wwork.tile([128, C * TAPS], F32, tag="wmix", name="wmix")
            ws = wwork.tile([128, C * TAPS], FP8, tag=f"wsign{h}", bufs=1,
                            name=f"wsign{h}")
            for ci in range(CIH):
                for k in range(K):
                    dst = wmix[:, ci * HCI:(ci + 1) * HCI]
                    nc.vector.scalar_tensor_tensor(
                        dst, wks_by_c[ci][k], rv[:, k:k + 1],
                        wks_by_c[ci][k] if k == 0 else dst,
                        mybir.AluOpType.mult,
                        mybir.AluOpType.bypass if k == 0 else
                        mybir.AluOpType.add)
                nc.scalar.sign(ws[:, ci * HCI:(ci + 1) * HCI],
                               wmix[:, ci * HCI:(ci + 1) * HCI])
            return ws, wmix

        # |real_w| row-sums + scale*alpha combine on DVE
        def reduce_half(h, wmix):
            absum = consts.tile([128, 1], F32, tag=f"ab{h}", name=f"ab{h}")
            nc.vector.tensor_reduce(absum, wmix, mybir.AxisListType.X,
                                    mybir.AluOpType.add,
                                    apply_absolute_value=True)
            nc.vector.scalar_tensor_tensor(
                scale_alpha[h], absum, 1.0 / (C * TAPS), alpha_sb[h],
                mybir.AluOpType.mult, mybir.AluOpType.mult)

        # --- transpose one co-half's sign-weights into wT -------------------
        # Staged across three PSUM regions so the matmuls run back-to-back
        # with few ACT copies; stage order frees conv's psum tiles early.
        def transpose_half(h, wsgn, cpsum):
            wsv = wsgn.rearrange("p (ci t) -> p ci t", t=TAPS)
            if h == 0:
                # ps0 first (frees conv00's first psum tile early)
                stages = [("ps0", 4 * 512, 8, 9, 1), ("ps1", 3 * 512, 0, 6, 2),
                          ("tps", 512, 6, 8, 1)]
            else:
                # ps1 last: it must wait for the previous conv's ptile-B
                # evacuation, which lands latest
                stages = [("ps0", 4 * 512, 0, 2, 1), ("tps", 512, 2, 4, 1),
                          ("ps1", 3 * 512, 4, 9, 2)]
            for tag, width, ta, tb, ncopy in stages:
                tp = cpsum.tile([128, width], F32, tag=tag, bufs=1,
                                name=f"t{tag}")
                for i, (tap, ci) in enumerate(
                        [(t, c) for t in range(ta, tb) for c in range(CIH)]):
                    nc.tensor.matmul(
                        tp[:, i * 128:(i + 1) * 128],
                        wsv[:, ci * 128:(ci + 1) * 128, tap], ident_8,
                        start=True, stop=True)
                nt = tb - ta
                for ic in range(ncopy):
                    ca = ta + ic * nt // ncopy
                    cb = ta + (ic + 1) * nt // ncopy
                    o0 = (ca - ta) * CIH * 128
                    nc.scalar.copy(
                        wT[:, ca:cb, h, :, :],
                        tp[:, o0:o0 + (cb - ca) * CIH * 128].rearrange(
                            "p (t ci co) -> p t ci co", t=cb - ca, co=128))

        # --- load + sign one image into its padded plane ------------------
        def load(b, part=None):
            # upper rows of both ci-halves first: the first conv ptile only
            # needs rows < XSPL, so it must not queue behind the lower rows
            tiles = []
            for (r0, r1) in ((0, XSPL), (XSPL, H)):
                if part == "lower" and r0 == 0:
                    continue
                if part == "upper" and r0 != 0:
                    continue
                for s in range(CIH):
                    xs = xin.tile([128, (r1 - r0) * W], F32, tag="xsb",
                                  name="xsb")
                    nc.sync.dma_start(
                        out=xs,
                        in_=x_d.ap()[b, s * 128:(s + 1) * 128, r0:r1]
                        .rearrange("p a b -> p (a b)"))
                    tiles.append((s, r0, r1, xs))
            return tiles

        def sign(b, tiles):
            xp = xpad[b % 2]
            for (s, r0, r1, xs) in tiles:
                dst = xp[:, s, GO + (r0 + 1) * PW:GO + (r1 + 1) * PW] \
                    .rearrange("p (y x) -> p y x", x=PW)[:, :, 0:W]
                nc.scalar.sign(dst, xs.rearrange("p (y x) -> p y x", x=W))

        # --- conv for one (image, co-half) --------------------------------
        def conv(b, h, cpsum, tail=False):
            xp = xpad[b % 2]
            osb = outp.tile([128, H * W], BF16, tag="osb", name="osb")
            # consume taps in the order the transpose stages produce them
            tap_order = [8, 0, 1, 2, 3, 4, 5, 6, 7] if h == 0 \
                else list(range(TAPS))
            c0 = 0
            for t, nch in enumerate(PT_CHUNKS):
                ps = cpsum.tile([128, nch * 512], F32, tag=f"ps{t}", bufs=1,
                                name=f"ps{t}")
                for itap, tap in enumerate(tap_order):
                    dy, dx = tap // KS - 1, tap % KS - 1
                    lhsT = wT[:, tap, h, :, :]
                    for j in range(nch):
                        c = c0 + j
                        off = GO + (RPC * c + dy + 1) * PW + dx
                        nc.tensor.matmul(
                            ps[:, j * 512:j * 512 + CHUNK], lhsT,
                            xp[:, :, off:off + CHUNK],
                            start=(itap == 0), stop=(itap == TAPS - 1),
                            perf_mode=mybir.MatmulPerfMode.DoubleRow)
                # the very last ptile of the kernel drains in single-chunk
                # pieces so the final evac and output DMA pipeline
                pieces = ([(j, j + 1) for j in range(nch)] if (tail and t == 1)
                          else [(0, nch)])
                for ja, jb in pieces:
                    src = ps.rearrange("p (c e) -> p c e", e=512)[
                        :, ja:jb, 0:CHUNK].rearrange(
                        "p c (r x) -> p c r x", x=PW)[:, :, :, 0:W]
                    dst = osb.rearrange("p (y x) -> p y x", x=W)[
                        :, (c0 + ja) * RPC:(c0 + jb) * RPC, :].rearrange(
                        "p (c r) x -> p c r x", r=RPC)
                    # PSUM evacuation on ACT with fused scale*alpha
                    nc.scalar.activation(dst, src,
                                         mybir.ActivationFunctionType.Copy,
                                         bias=0.0, scale=scale_alpha[h])
                    # per-piece output DMA on the ACT ring
                    nc.scalar.dma_start(
                        out=o_d.ap()[b, h * 128:(h + 1) * 128,
                                     (c0 + ja) * RPC:(c0 + jb) * RPC, :]
                        .rearrange("p a b -> p (a b)"),
                        in_=osb[:, (c0 + ja) * RPC * W:(c0 + jb) * RPC * W])
                c0 += nch

        # --- schedule ------------------------------------------------------
        with tc.tile_pool(name="cpsum", bufs=1, space="PSUM") as cpsum:
            bcast_alpha(cpsum)
            # DMA ring order: w-h0/ci0, x0 upper rows, w-h0/ci1, w-h1/ci0,
            # x0 lower rows, w-h1/ci1, then the image stream.
            wk_h0 = [dma_w(0, 0)]
            xt0u = load(0, part="upper")
            wk_h0.append(dma_w(0, 1))
            wk_h1 = [dma_w(1, 0)]
            xt0l = load(0, part="lower")
            wk_h1.append(dma_w(1, 1))
            sign(0, xt0u)
            # HAM warmups: 8 on the first c0 chunk (~5us), then 2 each on
            # later-landing chunks to bridge to the transposes
            warm(cpsum, wk_h0[0][0], 8)
            warm(cpsum, wk_h0[1][0], 2)
            warm(cpsum, wk_h0[1][3], 2)
            ws0, wm0 = prep_half(0, wk_h0)
            transpose_half(0, ws0, cpsum)
            reduce_half(0, wm0)
            sign(0, xt0l)
            conv(0, 0, cpsum)
            ws1, wm1 = prep_half(1, wk_h1)
            transpose_half(1, ws1, cpsum)
            reduce_half(1, wm1)
            xt1 = load(1)
            sign(1, xt1)
            conv(0, 1, cpsum)
            for b in range(1, BL):
                if b + 1 < BL:
                    xt = load(b + 1)   # prefetch ahead of this image's evacs
                    sign(b + 1, xt)
                conv(b, 0, cpsum)
                conv(b, 1, cpsum, tail=(b == BL - 1))
    nc.compile()
    return nc


def _get_nc():
    if "nc" not in _cache:
        _cache["nc"] = _build()
    return _cache["nc"]


def run(inputs, trace=False):
    nc = _get_nc()
    x = np.ascontiguousarray(inputs["x"], dtype=np.float32)
    in_maps = [
        {
            "x": x[c * BL:(c + 1) * BL],
            "weights": np.ascontiguousarray(inputs["weights"], np.float32),
            "RV": np.ascontiguousarray(inputs["RV"], np.float32),
            "alpha": np.ascontiguousarray(inputs["alpha"], np.float32),
        }
        for c in range(NCORES)
    ]
    res = run_bass_kernel_spmd(nc, in_maps, core_ids=list(range(NCORES)),
                               trace=trace)
    out = np.concatenate([np.asarray(r["out"]).astype(np.float32)
                          for r in res.results], axis=0)
    return out, res


def kernel(**inputs) -> np.ndarray:
    out, _ = run(inputs, trace=False)
    return out
